# revision 6
# baseline (speedup 1.0000x reference)
"""Trainium2 Bass kernel for the AttentionUnit GNN message-passing block.

Math
----
The nn.Module lifts scalars to `channel` dims with rank-1 weights, so the
whole block collapses to per-batch scalar attention:

    s[b,i,j] = alpha * e[b,i] * v[b,j],     alpha = w_g . w_f
    E = exp(s);  cs[j] = sum_i E[i,j];  rs[i] = sum_j E[i,j]
    out_v = v + beta  * E   @ (v / cs),     beta  = w_h . w_m
    out_e = e + gamma * E^T @ (e / rs),     gamma = w_l . w_n

Since |s| <= m ~ 0.3 (data-dependent, computed at runtime), exp(s) is
replaced by a degree-DEG Chebyshev-interpolated polynomial, which makes E a
rank-(DEG+1) matrix  E = sum_k c_k (e^k)(v^k)^T  that is never materialized.

Per core (64 batch rows, stacked X = [v rows; e rows] on 128 partitions):
  - bf16 power chain P_k = X^k with fp32 row-sum accumulators R_k
  - den = sum_k diag(c_k * swap(R_k)) @ P_k   -- PE matmuls accumulating in
    PSUM; diag tiles are built with one tensor-scalar op on a 0/1 identity
  - Y_k = sum_j P_{k+1} * (1/den)             -- DVE fused mul+reduce
  - O_swapped = sum_k permdiag(g_k) @ P_k     -- PE matmuls; permdiag(g) =
    perm-matrix * per-partition scalars, so the half-swap of the correction
    is folded into the same accumulation
  - out = X + O_swapped
Half-swaps of the small [128,k] scalar blocks use one tiny fp32 PE matmul
with a permutation matrix.

The polynomial coefficients depend on the input data, so they are passed as
small input tensors -> the compiled NEFF is input-independent and cached.
"""

import os
from contextlib import ExitStack

import numpy as np

import concourse.bass as bass
import concourse.tile as tile
from concourse import bacc, mybir
from concourse.bass_utils import run_bass_kernel_spmd

B = 512          # batch
D = 512          # dim
N_CORES = 8
BC = B // N_CORES  # 64 batch rows per core
P = 128            # partitions: [v (0..63); e (64..127)]
DEG = int(os.environ.get("ATTN_KERNEL_DEG", "5"))

f32 = mybir.dt.float32
bf16 = mybir.dt.bfloat16
MULT = mybir.AluOpType.mult
ADD = mybir.AluOpType.add
NCOL = 8  # padded column count for the R/Y scalar blocks


def _build_program(deg: int):
    """Build + compile the single-core Tile program (same NEFF on all 8 cores)."""
    nc = bacc.Bacc(
        "TRN2",
        target_bir_lowering=False,
        debug=False,
        enable_asserts=False,
    )

    xv_d = nc.dram_tensor("xv", [BC, D], f32, kind="ExternalInput")
    xe_d = nc.dram_tensor("xe", [BC, D], f32, kind="ExternalInput")
    # coefs columns: [0] = c_0 * D
    #                [1 : deg+1]          = c_k (k=1..deg)   for the den diags
    #                [deg+1 : 2*deg+2]    = swapped-cout * c_k (k=0..deg)
    coefs_d = nc.dram_tensor("coefs", [P, 2 * deg + 2], f32, kind="ExternalInput")
    permf_d = nc.dram_tensor("permf", [P, P], f32, kind="ExternalInput")
    permb_d = nc.dram_tensor("permb", [P, P], bf16, kind="ExternalInput")
    iden_d = nc.dram_tensor("iden", [P, P], bf16, kind="ExternalInput")
    ov_d = nc.dram_tensor("out_v", [BC, D], f32, kind="ExternalOutput")
    oe_d = nc.dram_tensor("out_e", [BC, D], f32, kind="ExternalOutput")

    with tile.TileContext(nc) as tc, ExitStack() as ctx:
        big = ctx.enter_context(tc.tile_pool(name="big", bufs=1))
        scr = ctx.enter_context(tc.tile_pool(name="scr", bufs=2))
        small = ctx.enter_context(tc.tile_pool(name="small", bufs=1))
        ps_s = ctx.enter_context(
            tc.tile_pool(name="ps_s", bufs=2, space=bass.MemorySpace.PSUM)
        )
        ps_b = ctx.enter_context(
            tc.tile_pool(name="ps_b", bufs=2, space=bass.MemorySpace.PSUM)
        )

        # ---- constants in, ACT Square table warm-up ----
        coefs = small.tile([P, 2 * deg + 2], f32, name="coefs_t")
        nc.scalar.dma_start(coefs[:], coefs_d[:])
        permf = small.tile([P, P], f32, name="permf_t")
        nc.sync.dma_start(permf[:], permf_d[:])
        permb = small.tile([P, P], bf16, name="permb_t")
        nc.scalar.dma_start(permb[:], permb_d[:])
        iden = small.tile([P, P], bf16, name="iden_t")
        nc.gpsimd.dma_start(iden[:], iden_d[:])
        warm = small.tile([P, 1], bf16, name="warm")
        nc.scalar.activation(warm[:], iden[:, 0:1],
                             mybir.ActivationFunctionType.Square)
        ones = big.tile([P, D], bf16, name="ones")
        nc.gpsimd.memset(ones[:], 1.0)

        X = big.tile([P, D], f32, name="X")
        nc.sync.dma_start(X[0:BC, :], xv_d[:])
        nc.gpsimd.dma_start(X[BC:P, :], xe_d[:])

        # ---- bf16 powers P_k = X^k, fp32 row-sums R_k (fused accumulators) --
        Rall = small.tile([P, NCOL], f32, name="Rall")
        Xb = big.tile([P, D], bf16, name="Xb")
        nc.vector.tensor_scalar(
            out=Xb[:], in0=X[:], scalar1=1.0, scalar2=0.0, op0=MULT, op1=ADD,
            accum_out=Rall[:, 0:1],
        )
        Pw = {1: Xb}
        for k in range(2, deg + 2):
            Pw[k] = big.tile([P, D], bf16, name=f"P{k}")
            accum = Rall[:, k - 1 : k] if k <= deg else None
            if k % 2 == 0:
                nc.scalar.activation(
                    Pw[k][:], Pw[k // 2][:],
                    mybir.ActivationFunctionType.Square, accum_out=accum,
                )
            else:
                nc.vector.scalar_tensor_tensor(
                    out=Pw[k][:], in0=Pw[k - 1][:], scalar=1.0, in1=Xb[:],
                    op0=MULT, op1=MULT, accum_out=accum,
                )

        # ---- den = sum_k diag(c_k * swap(R_k)) @ P_k  (+ c_0*D) ----
        Rs = small.tile([P, NCOL], f32, name="Rs")
        nc.vector.tensor_tensor(
            out=Rs[:, 0:deg], in0=Rall[:, 0:deg], in1=coefs[:, 1 : deg + 1],
            op=MULT,
        )
        rsw_ps = ps_s.tile([P, NCOL], f32, name="rsw_ps", tag="psw")
        nc.tensor.matmul(rsw_ps[:, 0:deg], permf[:], Rs[:, 0:deg],
                         start=True, stop=True)
        rsw = small.tile([P, NCOL], f32, name="rsw")
        nc.vector.tensor_copy(rsw[:, 0:deg], rsw_ps[:, 0:deg])

        den_ps = ps_b.tile([P, D], f32, name="den_ps", tag="acc")
        for k in range(1, deg + 1):
            dg = scr.tile([P, P], bf16, name=f"dg{k}", tag="dg")
            nc.scalar.mul(dg[:], iden[:], rsw[:, k - 1 : k])
            nc.tensor.matmul(den_ps[:], dg[:], Pw[k][:],
                             start=(k == 1), stop=(k == deg))

        den = big.tile([P, D], f32, name="den")
        nc.vector.tensor_scalar(
            out=den[:], in0=den_ps[:], scalar1=coefs[:, 0:1], scalar2=None,
            op0=ADD,
        )

        # ---- rcp = 1/den (fast approx), bf16 copy for the Y reductions ----
        rcp = big.tile([P, D], f32, name="rcp")
        nc.vector.reciprocal_approx_fast(out=rcp[:], in_=den[:])
        rcpb = big.tile([P, D], bf16, name="rcpb")
        nc.vector.tensor_copy(rcpb[:], rcp[:])

        # ---- Y_k = sum_j P_{k+1} * rcp  (x/den reductions) ----
        Yall = small.tile([P, NCOL], f32, name="Yall")
        for k in range(0, deg + 1):
            q = scr.tile([P, D], bf16, name=f"q{k}", tag="q")
            nc.vector.scalar_tensor_tensor(
                out=q[:], in0=Pw[k + 1][:], scalar=1.0, in1=rcpb[:],
                op0=MULT, op1=MULT, accum_out=Yall[:, k : k + 1],
            )

        # ---- O_swapped = sum_k permdiag(coutc_k * swap(Y_k)) @ P_k ----
        Ys = small.tile([P, NCOL], f32, name="Ys")
        nc.vector.tensor_tensor(
            out=Ys[:, 0 : deg + 1], in0=Yall[:, 0 : deg + 1],
            in1=coefs[:, deg + 1 : 2 * deg + 2], op=MULT,
        )
        ysw_ps = ps_s.tile([P, NCOL], f32, name="ysw_ps", tag="psw")
        nc.tensor.matmul(ysw_ps[:, 0 : deg + 1], permf[:], Ys[:, 0 : deg + 1],
                         start=True, stop=True)
        ysw = small.tile([P, NCOL], f32, name="ysw")
        nc.vector.tensor_copy(ysw[:, 0 : deg + 1], ysw_ps[:, 0 : deg + 1])

        osw_ps = ps_b.tile([P, D], f32, name="osw_ps", tag="acc")
        for k in range(0, deg + 1):
            pd = scr.tile([P, P], bf16, name=f"pd{k}", tag="dg")
            nc.scalar.mul(pd[:], permb[:], ysw[:, k : k + 1])
            rhs = ones if k == 0 else Pw[k]
            nc.tensor.matmul(osw_ps[:], pd[:], rhs[:],
                             start=(k == 0), stop=(k == deg))

        # ---- out = X + O_swapped ----
        OUT = big.tile([P, D], f32, name="OUT")
        nc.vector.tensor_tensor(out=OUT[:], in0=X[:], in1=osw_ps[:], op=ADD)

        nc.sync.dma_start(ov_d[:], OUT[0:BC, :])
        nc.gpsimd.dma_start(oe_d[:], OUT[BC:P, :])

    nc.compile()
    return nc


_PROGRAMS: dict[int, object] = {}


def _get_program(deg: int):
    if deg not in _PROGRAMS:
        _PROGRAMS[deg] = _build_program(deg)
    return _PROGRAMS[deg]


def _host_constants(v, e, w_f, w_g, w_h, w_l, w_m, w_n, deg):
    alpha = float(np.dot(w_g.astype(np.float64), w_f.astype(np.float64)))
    beta = float(np.dot(w_h.astype(np.float64), w_m.astype(np.float64)))
    gamma = float(np.dot(w_l.astype(np.float64), w_n.astype(np.float64)))

    # per-batch bound on |s| = |alpha * e_i * v_j|
    m = abs(alpha) * float(
        (np.abs(e).max(axis=1) * np.abs(v).max(axis=1)).max()
    )
    m = max(m * 1.02, 1e-6)

    cheb = np.polynomial.chebyshev.Chebyshev.interpolate(np.exp, deg, domain=[-m, m])
    q = cheb.convert(kind=np.polynomial.polynomial.Polynomial).coef
    q = np.concatenate([q, np.zeros(deg + 1 - len(q))])
    c = np.array([q[k] * alpha**k for k in range(deg + 1)], dtype=np.float64)

    coefs = np.zeros((P, 2 * deg + 2), dtype=np.float32)
    coefs[:, 0] = c[0] * D
    coefs[:, 1 : deg + 1] = c[1:]
    # permdiag scale: applied BEFORE the half-swap, so use swapped cout
    # (v-half rows feed e-half outputs -> beta there, and vice versa)
    cout_swapped = np.where(np.arange(P) < BC, beta, gamma)
    for k in range(deg + 1):
        coefs[:, deg + 1 + k] = cout_swapped * c[k]

    perm = np.zeros((P, P), dtype=np.float32)
    mm = np.arange(P)
    perm[(mm + BC) % P, mm] = 1.0
    iden = np.eye(P, dtype=np.float32)
    return coefs, perm, iden


def _run(inputs: dict, trace: bool = False):
    import ml_dtypes

    v = np.ascontiguousarray(np.asarray(inputs["v_input"], dtype=np.float32))
    e = np.ascontiguousarray(np.asarray(inputs["e_input"], dtype=np.float32))
    assert v.shape == (B, D) and e.shape == (B, D), (v.shape, e.shape)
    ws = {k: np.asarray(inputs[k], dtype=np.float32)
          for k in ("w_f", "w_g", "w_h", "w_l", "w_m", "w_n")}

    coefs, perm, iden = _host_constants(
        v, e, ws["w_f"], ws["w_g"], ws["w_h"], ws["w_l"], ws["w_m"], ws["w_n"], DEG
    )
    perm_bf = perm.astype(ml_dtypes.bfloat16)
    iden_bf = iden.astype(ml_dtypes.bfloat16)

    nc = _get_program(DEG)
    in_maps = []
    for cidx in range(N_CORES):
        sl = slice(cidx * BC, (cidx + 1) * BC)
        in_maps.append(
            {
                "xv": np.ascontiguousarray(v[sl]),
                "xe": np.ascontiguousarray(e[sl]),
                "coefs": coefs,
                "permf": perm,
                "permb": perm_bf,
                "iden": iden_bf,
            }
        )

    res = run_bass_kernel_spmd(nc, in_maps, list(range(N_CORES)), trace=trace)
    out_v = np.concatenate([res.results[c]["out_v"] for c in range(N_CORES)], axis=0)
    out_e = np.concatenate([res.results[c]["out_e"] for c in range(N_CORES)], axis=0)
    return (out_v, out_e), res


def kernel(**inputs):
    (out_v, out_e), _ = _run(inputs, trace=False)
    return out_v, out_e


# revision 7
# speedup vs baseline: 1.1513x; 1.1513x over previous
"""Trainium2 Bass kernel for the AttentionUnit GNN message-passing block.

Math
----
The nn.Module lifts scalars to `channel` dims with rank-1 weights, so the
whole block collapses to per-batch scalar attention:

    s[b,i,j] = alpha * e[b,i] * v[b,j],     alpha = w_g . w_f
    E = exp(s);  cs[j] = sum_i E[i,j];  rs[i] = sum_j E[i,j]
    out_v = v + beta  * E   @ (v / cs),     beta  = w_h . w_m
    out_e = e + gamma * E^T @ (e / rs),     gamma = w_l . w_n

Since |s| <= m ~ 0.3 (data-dependent, computed at runtime), exp(s) is
replaced by a degree-DEG Chebyshev-interpolated polynomial, which makes E a
rank-(DEG+1) matrix  E = sum_k c_k (e^k)(v^k)^T  that is never materialized:

    den = sum_k c_k A'_k X^k        (cs on the v-half, rs on the e-half)
    Y_k = sum_j X^{k+1} / den       (fused multiply-reduce against 1/den)
    out = swap(X) + sum_k g_k X^k   (g_k = swapped, scaled Y_k)

Layout: pure data parallel over 8 cores, 64 batch rows per core, stacked as
X = [v rows (partitions 0..63); e rows (64..127)] so every op handles both
sides at once. Cross-half swaps of [128,few] scalar blocks use two tiny
SBUF->SBUF DMAs; the final residual add uses a pre-swapped copy of the
input DMA'd at start, so no PE/transpose work is needed anywhere.

The polynomial coefficients depend on the input data, so they are passed as
small input tensors -> the compiled NEFF is input-independent and cached.
"""

import os
from contextlib import ExitStack

import numpy as np

import concourse.bass as bass
import concourse.tile as tile
from concourse import bacc, mybir
from concourse.bass_utils import run_bass_kernel_spmd

B = 512          # batch
D = 512          # dim
N_CORES = 8
BC = B // N_CORES  # 64 batch rows per core
P = 128            # partitions: [v (0..63); e (64..127)]
DEG = int(os.environ.get("ATTN_KERNEL_DEG", "4"))

f32 = mybir.dt.float32
MULT = mybir.AluOpType.mult
ADD = mybir.AluOpType.add
NCOL = 8  # padded column count for the R/Y scalar blocks
AF = mybir.ActivationFunctionType


def _build_program(deg: int):
    """Build + compile the single-core Tile program (same NEFF on all 8 cores)."""
    assert deg == 4, "tree-structured chains below are written for deg=4"
    nc = bacc.Bacc(
        "TRN2",
        target_bir_lowering=False,
        debug=False,
        enable_asserts=False,
    )

    xv_d = nc.dram_tensor("xv", [BC, D], f32, kind="ExternalInput")
    xe_d = nc.dram_tensor("xe", [BC, D], f32, kind="ExternalInput")
    # coefs columns: [0] = c_0 * D
    #                [1 : deg+1]       = c_k (k=1..deg)            (den scale)
    #                [deg+1 : 2deg+2]  = swapped-cout * c_k (k=0..deg)
    coefs_d = nc.dram_tensor("coefs", [P, 2 * deg + 2], f32, kind="ExternalInput")
    ov_d = nc.dram_tensor("out_v", [BC, D], f32, kind="ExternalOutput")
    oe_d = nc.dram_tensor("out_e", [BC, D], f32, kind="ExternalOutput")

    with tile.TileContext(nc) as tc, ExitStack() as ctx:
        big = ctx.enter_context(tc.tile_pool(name="big", bufs=1))
        scr = ctx.enter_context(tc.tile_pool(name="scr", bufs=2))
        small = ctx.enter_context(tc.tile_pool(name="small", bufs=1))

        # ---- inputs: X and its half-swapped copy (for the final residual) --
        X = big.tile([P, D], f32, name="X")
        nc.sync.dma_start(X[0:BC, :], xv_d[:])
        nc.scalar.dma_start(X[BC:P, :], xe_d[:])
        Xs = big.tile([P, D], f32, name="Xs")
        nc.sync.dma_start(Xs[BC:P, :], xv_d[:])
        nc.scalar.dma_start(Xs[0:BC, :], xe_d[:])
        coefs = small.tile([P, 2 * deg + 2], f32, name="coefs_t")
        nc.gpsimd.dma_start(coefs[:], coefs_d[:])

        # ---- ACT table warm-up (Square + Identity), off critical path ----
        warm = small.tile([P, 1], f32, name="warm")
        nc.scalar.activation(warm[:], coefs[:, 0:1], AF.Square)
        warm2 = small.tile([P, 1], f32, name="warm2")
        nc.scalar.activation(warm2[:], coefs[:, 0:1], AF.Identity,
                             bias=0.0, scale=1.0)

        # ---- powers P_k = X^k with fused row-sums R_k ----
        Rall = small.tile([P, NCOL], f32, name="Rall")
        nc.vector.tensor_reduce(Rall[:, 0:1], X[:], axis=mybir.AxisListType.X,
                                op=ADD)
        P2 = big.tile([P, D], f32, name="P2")
        nc.scalar.activation(P2[:], X[:], AF.Square, accum_out=Rall[:, 1:2])
        P3 = big.tile([P, D], f32, name="P3")
        nc.vector.scalar_tensor_tensor(
            out=P3[:], in0=P2[:], scalar=1.0, in1=X[:],
            op0=MULT, op1=MULT, accum_out=Rall[:, 2:3],
        )
        P4 = big.tile([P, D], f32, name="P4")
        nc.scalar.activation(P4[:], P2[:], AF.Square, accum_out=Rall[:, 3:4])
        P5 = big.tile([P, D], f32, name="P5")
        nc.vector.scalar_tensor_tensor(
            out=P5[:], in0=P4[:], scalar=1.0, in1=X[:], op0=MULT, op1=MULT,
        )
        Pw = {1: X, 2: P2, 3: P3, 4: P4, 5: P5}

        # ---- b_k = c_k * swap(R_k): scale then two tiny SBUF->SBUF DMAs ----
        Rs = small.tile([P, NCOL], f32, name="Rs")
        nc.vector.tensor_tensor(
            out=Rs[:, 0:deg], in0=Rall[:, 0:deg], in1=coefs[:, 1 : deg + 1],
            op=MULT,
        )
        Bt = small.tile([P, NCOL], f32, name="Bt")
        nc.sync.dma_start(Bt[0:BC, 0:deg], Rs[BC:P, 0:deg])
        nc.scalar.dma_start(Bt[BC:P, 0:deg], Rs[0:BC, 0:deg])

        # ---- den = cd0 + sum_k b_k X^k  (tree: ACT leaves, DVE joins) ----
        tA = scr.tile([P, D], f32, name="tA", tag="t")
        nc.scalar.activation(tA[:], X[:], AF.Identity,
                             bias=coefs[:, 0:1], scale=Bt[:, 0:1])
        tB = scr.tile([P, D], f32, name="tB", tag="t")
        nc.scalar.mul(tB[:], P4[:], Bt[:, 3:4])
        tC = scr.tile([P, D], f32, name="tC", tag="u")
        nc.vector.scalar_tensor_tensor(
            out=tC[:], in0=P2[:], scalar=Bt[:, 1:2], in1=tA[:],
            op0=MULT, op1=ADD,
        )
        tD = scr.tile([P, D], f32, name="tD", tag="u")
        nc.vector.scalar_tensor_tensor(
            out=tD[:], in0=P3[:], scalar=Bt[:, 2:3], in1=tB[:],
            op0=MULT, op1=ADD,
        )
        den = big.tile([P, D], f32, name="den")
        nc.vector.tensor_tensor(out=den[:], in0=tC[:], in1=tD[:], op=ADD)

        # ---- Y_k = sum_j X^{k+1} / den ----
        rcp = big.tile([P, D], f32, name="rcp")
        nc.vector.reciprocal_approx_fast(out=rcp[:], in_=den[:])
        Yall = small.tile([P, NCOL], f32, name="Yall")
        for k in range(0, deg + 1):
            q = scr.tile([P, D], f32, name=f"q{k}", tag="q")
            nc.vector.scalar_tensor_tensor(
                out=q[:], in0=Pw[k + 1][:], scalar=1.0, in1=rcp[:],
                op0=MULT, op1=MULT, accum_out=Yall[:, k : k + 1],
            )

        # ---- g_k = coutc_k * swap(Y_k) ----
        Ys = small.tile([P, NCOL], f32, name="Ys")
        nc.vector.tensor_tensor(
            out=Ys[:, 0 : deg + 1], in0=Yall[:, 0 : deg + 1],
            in1=coefs[:, deg + 1 : 2 * deg + 2], op=MULT,
        )
        Gt = small.tile([P, NCOL], f32, name="Gt")
        nc.sync.dma_start(Gt[0:BC, 0 : deg + 1], Ys[BC:P, 0 : deg + 1])
        nc.scalar.dma_start(Gt[BC:P, 0 : deg + 1], Ys[0:BC, 0 : deg + 1])

        # ---- OUT = swap(X) + g_0 + sum_k g_k X^k ----
        uA = scr.tile([P, D], f32, name="uA", tag="t")
        nc.scalar.activation(uA[:], X[:], AF.Identity,
                             bias=Gt[:, 0:1], scale=Gt[:, 1:2])
        uC = scr.tile([P, D], f32, name="uC", tag="u")
        nc.vector.scalar_tensor_tensor(
            out=uC[:], in0=P2[:], scalar=Gt[:, 2:3], in1=uA[:],
            op0=MULT, op1=ADD,
        )
        uD = scr.tile([P, D], f32, name="uD", tag="t")
        nc.vector.scalar_tensor_tensor(
            out=uD[:], in0=P3[:], scalar=Gt[:, 3:4], in1=Xs[:],
            op0=MULT, op1=ADD,
        )
        uE = scr.tile([P, D], f32, name="uE", tag="u")
        nc.vector.scalar_tensor_tensor(
            out=uE[:], in0=P4[:], scalar=Gt[:, 4:5], in1=uD[:],
            op0=MULT, op1=ADD,
        )
        OUT = big.tile([P, D], f32, name="OUT")
        nc.vector.tensor_tensor(out=OUT[:], in0=uC[:], in1=uE[:], op=ADD)

        nc.sync.dma_start(ov_d[:], OUT[BC:P, :])
        nc.scalar.dma_start(oe_d[:], OUT[0:BC, :])

    nc.compile()
    return nc


_PROGRAMS: dict[int, object] = {}


def _get_program(deg: int):
    if deg not in _PROGRAMS:
        _PROGRAMS[deg] = _build_program(deg)
    return _PROGRAMS[deg]


def _host_constants(v, e, w_f, w_g, w_h, w_l, w_m, w_n, deg):
    alpha = float(np.dot(w_g.astype(np.float64), w_f.astype(np.float64)))
    beta = float(np.dot(w_h.astype(np.float64), w_m.astype(np.float64)))
    gamma = float(np.dot(w_l.astype(np.float64), w_n.astype(np.float64)))

    # per-batch bound on |s| = |alpha * e_i * v_j|
    m = abs(alpha) * float(
        (np.abs(e).max(axis=1) * np.abs(v).max(axis=1)).max()
    )
    m = max(m * 1.02, 1e-6)

    cheb = np.polynomial.chebyshev.Chebyshev.interpolate(np.exp, deg, domain=[-m, m])
    q = cheb.convert(kind=np.polynomial.polynomial.Polynomial).coef
    q = np.concatenate([q, np.zeros(deg + 1 - len(q))])
    c = np.array([q[k] * alpha**k for k in range(deg + 1)], dtype=np.float64)

    coefs = np.zeros((P, 2 * deg + 2), dtype=np.float32)
    coefs[:, 0] = c[0] * D
    coefs[:, 1 : deg + 1] = c[1:]
    # g-scale is applied BEFORE the half-swap, so use swapped cout:
    # v-half rows feed e-half outputs (beta there) and vice versa.
    cout_swapped = np.where(np.arange(P) < BC, beta, gamma)
    for k in range(deg + 1):
        coefs[:, deg + 1 + k] = cout_swapped * c[k]
    return coefs


def _run(inputs: dict, trace: bool = False):
    v = np.ascontiguousarray(np.asarray(inputs["v_input"], dtype=np.float32))
    e = np.ascontiguousarray(np.asarray(inputs["e_input"], dtype=np.float32))
    assert v.shape == (B, D) and e.shape == (B, D), (v.shape, e.shape)
    ws = {k: np.asarray(inputs[k], dtype=np.float32)
          for k in ("w_f", "w_g", "w_h", "w_l", "w_m", "w_n")}

    coefs = _host_constants(
        v, e, ws["w_f"], ws["w_g"], ws["w_h"], ws["w_l"], ws["w_m"], ws["w_n"], DEG
    )

    nc = _get_program(DEG)
    in_maps = []
    for cidx in range(N_CORES):
        sl = slice(cidx * BC, (cidx + 1) * BC)
        in_maps.append(
            {
                "xv": np.ascontiguousarray(v[sl]),
                "xe": np.ascontiguousarray(e[sl]),
                "coefs": coefs,
            }
        )

    res = run_bass_kernel_spmd(nc, in_maps, list(range(N_CORES)), trace=trace)
    out_v = np.concatenate([res.results[c]["out_v"] for c in range(N_CORES)], axis=0)
    out_e = np.concatenate([res.results[c]["out_e"] for c in range(N_CORES)], axis=0)
    return (out_v, out_e), res


def kernel(**inputs):
    (out_v, out_e), _ = _run(inputs, trace=False)
    return out_v, out_e


# revision 9
# speedup vs baseline: 1.1804x; 1.0253x over previous
"""Trainium2 Bass kernel for the AttentionUnit GNN message-passing block.

Math
----
The nn.Module lifts scalars to `channel` dims with rank-1 weights, so the
whole block collapses to per-batch scalar attention:

    s[b,i,j] = alpha * e[b,i] * v[b,j],     alpha = w_g . w_f
    E = exp(s);  cs[j] = sum_i E[i,j];  rs[i] = sum_j E[i,j]
    out_v = v + beta  * E   @ (v / cs),     beta  = w_h . w_m
    out_e = e + gamma * E^T @ (e / rs),     gamma = w_l . w_n

Since |s| <= m ~ 0.3 (data-dependent, computed at runtime), exp(s) is
replaced by a degree-DEG Chebyshev-interpolated polynomial, which makes E a
rank-(DEG+1) matrix  E = sum_k c_k (e^k)(v^k)^T  that is never materialized:

    den = sum_k c_k A'_k X^k        (cs on the v-half, rs on the e-half)
    Y_k = sum_j X^{k+1} / den       (fused multiply-reduce against 1/den)
    out = swap(X) + sum_k g_k X^k   (g_k = swapped, scaled Y_k)

Layout: pure data parallel over 8 cores, 64 batch rows per core, stacked as
X = [v rows (partitions 0..63); e rows (64..127)] so every op handles both
sides at once. Cross-half swaps of [128,few] scalar blocks use two tiny
SBUF->SBUF DMAs; the final residual add uses a pre-swapped copy of the
input DMA'd at start, so no PE/transpose work is needed anywhere.

The polynomial coefficients depend on the input data, so they are passed as
small input tensors -> the compiled NEFF is input-independent and cached.
"""

import os
from contextlib import ExitStack

import numpy as np

import concourse.bass as bass
import concourse.tile as tile
from concourse import bacc, mybir
from concourse.bass_utils import run_bass_kernel_spmd

B = 512          # batch
D = 512          # dim
N_CORES = 8
BC = B // N_CORES  # 64 batch rows per core
P = 128            # partitions: [v (0..63); e (64..127)]
DEG = int(os.environ.get("ATTN_KERNEL_DEG", "4"))

f32 = mybir.dt.float32
MULT = mybir.AluOpType.mult
ADD = mybir.AluOpType.add
NCOL = 8  # padded column count for the R/Y scalar blocks
AF = mybir.ActivationFunctionType


def _build_program(deg: int):
    """Build + compile the single-core Tile program (same NEFF on all 8 cores)."""
    assert deg == 4, "tree-structured chains below are written for deg=4"
    nc = bacc.Bacc(
        "TRN2",
        target_bir_lowering=False,
        debug=False,
        enable_asserts=False,
    )

    xv_d = nc.dram_tensor("xv", [BC, D], f32, kind="ExternalInput")
    xe_d = nc.dram_tensor("xe", [BC, D], f32, kind="ExternalInput")
    # coefs columns: [0] = c_0 * D
    #                [1 : deg+1]       = c_k (k=1..deg)            (den scale)
    #                [deg+1 : 2deg+2]  = swapped-cout * c_k (k=0..deg)
    coefs_d = nc.dram_tensor("coefs", [P, 2 * deg + 2], f32, kind="ExternalInput")
    ov_d = nc.dram_tensor("out_v", [BC, D], f32, kind="ExternalOutput")
    oe_d = nc.dram_tensor("out_e", [BC, D], f32, kind="ExternalOutput")

    with tile.TileContext(nc) as tc, ExitStack() as ctx:
        big = ctx.enter_context(tc.tile_pool(name="big", bufs=1))
        scr = ctx.enter_context(tc.tile_pool(name="scr", bufs=2))
        small = ctx.enter_context(tc.tile_pool(name="small", bufs=1))

        # ---- inputs: X and its half-swapped copy (for the final residual) --
        X = big.tile([P, D], f32, name="X")
        nc.sync.dma_start(X[0:BC, :], xv_d[:])
        nc.scalar.dma_start(X[BC:P, :], xe_d[:])
        Xs = big.tile([P, D], f32, name="Xs")
        nc.sync.dma_start(Xs[BC:P, :], xv_d[:])
        nc.scalar.dma_start(Xs[0:BC, :], xe_d[:])
        coefs = small.tile([P, 2 * deg + 2], f32, name="coefs_t")
        nc.gpsimd.dma_start(coefs[:], coefs_d[:])

        # ---- ACT table warm-up (Square + Identity), off critical path ----
        warm = small.tile([P, 1], f32, name="warm")
        nc.scalar.activation(warm[:], coefs[:, 0:1], AF.Square)
        warm2 = small.tile([P, 1], f32, name="warm2")
        nc.scalar.activation(warm2[:], coefs[:, 0:1], AF.Identity,
                             bias=0.0, scale=1.0)

        # ---- powers P_k = X^k with fused row-sums R_k ----
        Rall = small.tile([P, NCOL], f32, name="Rall")
        nc.vector.tensor_reduce(Rall[:, 0:1], X[:], axis=mybir.AxisListType.X,
                                op=ADD)
        P2 = big.tile([P, D], f32, name="P2")
        nc.scalar.activation(P2[:], X[:], AF.Square, accum_out=Rall[:, 1:2])
        P3 = big.tile([P, D], f32, name="P3")
        nc.vector.scalar_tensor_tensor(
            out=P3[:], in0=P2[:], scalar=1.0, in1=X[:],
            op0=MULT, op1=MULT, accum_out=Rall[:, 2:3],
        )
        P4 = big.tile([P, D], f32, name="P4")
        nc.scalar.activation(P4[:], P2[:], AF.Square, accum_out=Rall[:, 3:4])
        P5 = big.tile([P, D], f32, name="P5")
        nc.vector.scalar_tensor_tensor(
            out=P5[:], in0=P4[:], scalar=1.0, in1=X[:], op0=MULT, op1=MULT,
        )
        Pw = {1: X, 2: P2, 3: P3, 4: P4, 5: P5}

        # ---- b_k = c_k * swap(R_k): per-pair scale + tiny SBUF->SBUF DMA
        # swaps, fired as soon as each pair of row-sums exists so the swap
        # latency hides under the power chain ----
        Rs = small.tile([P, NCOL], f32, name="Rs")
        Bt = small.tile([P, NCOL], f32, name="Bt")
        nc.vector.tensor_tensor(
            out=Rs[:, 0:2], in0=Rall[:, 0:2], in1=coefs[:, 1:3], op=MULT,
        )
        nc.sync.dma_start(Bt[0:BC, 0:2], Rs[BC:P, 0:2])
        nc.scalar.dma_start(Bt[BC:P, 0:2], Rs[0:BC, 0:2])
        nc.vector.tensor_tensor(
            out=Rs[:, 2:deg], in0=Rall[:, 2:deg], in1=coefs[:, 3 : deg + 1],
            op=MULT,
        )
        nc.sync.dma_start(Bt[0:BC, 2:deg], Rs[BC:P, 2:deg])
        nc.scalar.dma_start(Bt[BC:P, 2:deg], Rs[0:BC, 2:deg])

        # ---- den = cd0 + sum_k b_k X^k (DVE chain, earliest-ready first) --
        dB = scr.tile([P, D], f32, name="dB", tag="t")
        nc.vector.tensor_scalar(
            out=dB[:], in0=P2[:], scalar1=Bt[:, 1:2], scalar2=coefs[:, 0:1],
            op0=MULT, op1=ADD,
        )
        dA = scr.tile([P, D], f32, name="dA", tag="u")
        nc.vector.scalar_tensor_tensor(
            out=dA[:], in0=X[:], scalar=Bt[:, 0:1], in1=dB[:],
            op0=MULT, op1=ADD,
        )
        dC = scr.tile([P, D], f32, name="dC", tag="t")
        nc.vector.scalar_tensor_tensor(
            out=dC[:], in0=P3[:], scalar=Bt[:, 2:3], in1=dA[:],
            op0=MULT, op1=ADD,
        )
        den = big.tile([P, D], f32, name="den")
        nc.vector.scalar_tensor_tensor(
            out=den[:], in0=P4[:], scalar=Bt[:, 3:4], in1=dC[:],
            op0=MULT, op1=ADD,
        )

        # ---- Y_k = sum_j X^{k+1} / den; per-pair scale + swap of the g's
        # interleaved so each swap hides under the remaining Q reductions ----
        rcp = big.tile([P, D], f32, name="rcp")
        nc.vector.reciprocal_approx_fast(out=rcp[:], in_=den[:])
        Yall = small.tile([P, NCOL], f32, name="Yall")
        Ys = small.tile([P, NCOL], f32, name="Ys")
        Gt = small.tile([P, NCOL], f32, name="Gt")

        def y_swap(lo, hi):
            nc.vector.tensor_tensor(
                out=Ys[:, lo:hi], in0=Yall[:, lo:hi],
                in1=coefs[:, deg + 1 + lo : deg + 1 + hi], op=MULT,
            )
            nc.sync.dma_start(Gt[0:BC, lo:hi], Ys[BC:P, lo:hi])
            nc.scalar.dma_start(Gt[BC:P, lo:hi], Ys[0:BC, lo:hi])

        for k in range(0, deg + 1):
            q = scr.tile([P, D], f32, name=f"q{k}", tag="q")
            nc.vector.scalar_tensor_tensor(
                out=q[:], in0=Pw[k + 1][:], scalar=1.0, in1=rcp[:],
                op0=MULT, op1=MULT, accum_out=Yall[:, k : k + 1],
            )
            if k == 1:
                y_swap(0, 2)
            elif k == 3:
                y_swap(2, 4)
        y_swap(deg, deg + 1)

        # ---- OUT = swap(X) + g_0 + sum_k g_k X^k ----
        uA = scr.tile([P, D], f32, name="uA", tag="t")
        nc.scalar.activation(uA[:], X[:], AF.Identity,
                             bias=Gt[:, 0:1], scale=Gt[:, 1:2])
        uC = scr.tile([P, D], f32, name="uC", tag="u")
        nc.vector.scalar_tensor_tensor(
            out=uC[:], in0=P2[:], scalar=Gt[:, 2:3], in1=uA[:],
            op0=MULT, op1=ADD,
        )
        uD = scr.tile([P, D], f32, name="uD", tag="t")
        nc.vector.scalar_tensor_tensor(
            out=uD[:], in0=P3[:], scalar=Gt[:, 3:4], in1=Xs[:],
            op0=MULT, op1=ADD,
        )
        uE = scr.tile([P, D], f32, name="uE", tag="q")
        nc.vector.scalar_tensor_tensor(
            out=uE[:], in0=P4[:], scalar=Gt[:, 4:5], in1=uD[:],
            op0=MULT, op1=ADD,
        )
        # split the final join so the first output DMAs fire earlier
        OUT = big.tile([P, D], f32, name="OUT")
        H = D // 2
        nc.vector.tensor_tensor(out=OUT[:, 0:H], in0=uC[:, 0:H],
                                in1=uE[:, 0:H], op=ADD)
        nc.sync.dma_start(ov_d[:, 0:H], OUT[BC:P, 0:H])
        nc.scalar.dma_start(oe_d[:, 0:H], OUT[0:BC, 0:H])
        nc.vector.tensor_tensor(out=OUT[:, H:D], in0=uC[:, H:D],
                                in1=uE[:, H:D], op=ADD)
        nc.sync.dma_start(ov_d[:, H:D], OUT[BC:P, H:D])
        nc.scalar.dma_start(oe_d[:, H:D], OUT[0:BC, H:D])

    nc.compile()
    return nc


_PROGRAMS: dict[int, object] = {}


def _get_program(deg: int):
    if deg not in _PROGRAMS:
        _PROGRAMS[deg] = _build_program(deg)
    return _PROGRAMS[deg]


def _host_constants(v, e, w_f, w_g, w_h, w_l, w_m, w_n, deg):
    alpha = float(np.dot(w_g.astype(np.float64), w_f.astype(np.float64)))
    beta = float(np.dot(w_h.astype(np.float64), w_m.astype(np.float64)))
    gamma = float(np.dot(w_l.astype(np.float64), w_n.astype(np.float64)))

    # per-batch bound on |s| = |alpha * e_i * v_j|
    m = abs(alpha) * float(
        (np.abs(e).max(axis=1) * np.abs(v).max(axis=1)).max()
    )
    m = max(m * 1.02, 1e-6)

    cheb = np.polynomial.chebyshev.Chebyshev.interpolate(np.exp, deg, domain=[-m, m])
    q = cheb.convert(kind=np.polynomial.polynomial.Polynomial).coef
    q = np.concatenate([q, np.zeros(deg + 1 - len(q))])
    c = np.array([q[k] * alpha**k for k in range(deg + 1)], dtype=np.float64)

    coefs = np.zeros((P, 2 * deg + 2), dtype=np.float32)
    coefs[:, 0] = c[0] * D
    coefs[:, 1 : deg + 1] = c[1:]
    # g-scale is applied BEFORE the half-swap, so use swapped cout:
    # v-half rows feed e-half outputs (beta there) and vice versa.
    cout_swapped = np.where(np.arange(P) < BC, beta, gamma)
    for k in range(deg + 1):
        coefs[:, deg + 1 + k] = cout_swapped * c[k]
    return coefs


def _run(inputs: dict, trace: bool = False):
    v = np.ascontiguousarray(np.asarray(inputs["v_input"], dtype=np.float32))
    e = np.ascontiguousarray(np.asarray(inputs["e_input"], dtype=np.float32))
    assert v.shape == (B, D) and e.shape == (B, D), (v.shape, e.shape)
    ws = {k: np.asarray(inputs[k], dtype=np.float32)
          for k in ("w_f", "w_g", "w_h", "w_l", "w_m", "w_n")}

    coefs = _host_constants(
        v, e, ws["w_f"], ws["w_g"], ws["w_h"], ws["w_l"], ws["w_m"], ws["w_n"], DEG
    )

    nc = _get_program(DEG)
    in_maps = []
    for cidx in range(N_CORES):
        sl = slice(cidx * BC, (cidx + 1) * BC)
        in_maps.append(
            {
                "xv": np.ascontiguousarray(v[sl]),
                "xe": np.ascontiguousarray(e[sl]),
                "coefs": coefs,
            }
        )

    res = run_bass_kernel_spmd(nc, in_maps, list(range(N_CORES)), trace=trace)
    out_v = np.concatenate([res.results[c]["out_v"] for c in range(N_CORES)], axis=0)
    out_e = np.concatenate([res.results[c]["out_e"] for c in range(N_CORES)], axis=0)
    return (out_v, out_e), res


def kernel(**inputs):
    (out_v, out_e), _ = _run(inputs, trace=False)
    return out_v, out_e


# revision 11
# speedup vs baseline: 1.3614x; 1.1534x over previous
"""Trainium2 Bass kernel for the AttentionUnit GNN message-passing block.

Math
----
The nn.Module lifts scalars to `channel` dims with rank-1 weights, so the
whole block collapses to per-batch scalar attention:

    s[b,i,j] = alpha * e[b,i] * v[b,j],     alpha = w_g . w_f
    E = exp(s);  cs[j] = sum_i E[i,j];  rs[i] = sum_j E[i,j]
    out_v = v + beta  * E   @ (v / cs),     beta  = w_h . w_m
    out_e = e + gamma * E^T @ (e / rs),     gamma = w_l . w_n

Since |s| <= m ~ 0.3 (data-dependent, computed at runtime), exp(s) is
replaced by a degree-DEG Chebyshev-interpolated polynomial, which makes E a
rank-(DEG+1) matrix  E = sum_k c_k (e^k)(v^k)^T  that is never materialized:

    den = sum_k c_k A'_k X^k        (cs on the v-half, rs on the e-half)
    Y_k = sum_j X^{k+1} / den       (fused multiply-reduce against 1/den)
    out = swap(X) + sum_k g_k X^k   (g_k = swapped, scaled Y_k)

Layout: pure data parallel over 8 cores, 64 batch rows per core, stacked as
X = [v rows (partitions 0..63); e rows (64..127)] so every op handles both
sides at once. Cross-half swaps of [128,few] scalar blocks use two tiny
SBUF->SBUF DMAs; the final residual add uses a pre-swapped copy of the
input DMA'd at start, so no PE/transpose work is needed anywhere.

The polynomial coefficients depend on the input data, so they are passed as
small input tensors -> the compiled NEFF is input-independent and cached.
"""

import os
from contextlib import ExitStack

import numpy as np

import concourse.bass as bass
import concourse.tile as tile
from concourse import bacc, mybir
from concourse.bass_utils import run_bass_kernel_spmd

B = 512          # batch
D = 512          # dim
N_CORES = 8
BC = B // N_CORES  # 64 batch rows per core
P = 128            # partitions: [v (0..63); e (64..127)]
DEG = int(os.environ.get("ATTN_KERNEL_DEG", "4"))

f32 = mybir.dt.float32
MULT = mybir.AluOpType.mult
ADD = mybir.AluOpType.add
NCOL = 8  # padded column count for the R/Y scalar blocks
AF = mybir.ActivationFunctionType


def _build_program(deg: int):
    """Build + compile the single-core Tile program (same NEFF on all 8 cores)."""
    assert deg == 4, "tree-structured chains below are written for deg=4"
    nc = bacc.Bacc(
        "TRN2",
        target_bir_lowering=False,
        debug=False,
        enable_asserts=False,
    )

    xv_d = nc.dram_tensor("xv", [BC, D], f32, kind="ExternalInput")
    xe_d = nc.dram_tensor("xe", [BC, D], f32, kind="ExternalInput")
    # coefs columns: [0] = c_0 * D
    #                [1 : deg+1]       = c_k (k=1..deg)            (den scale)
    #                [deg+1 : 2deg+2]  = swapped-cout * c_k (k=0..deg)
    coefs_d = nc.dram_tensor("coefs", [P, 2 * deg + 2], f32, kind="ExternalInput")
    ov_d = nc.dram_tensor("out_v", [BC, D], f32, kind="ExternalOutput")
    oe_d = nc.dram_tensor("out_e", [BC, D], f32, kind="ExternalOutput")

    with tile.TileContext(nc) as tc, ExitStack() as ctx:
        big = ctx.enter_context(tc.tile_pool(name="big", bufs=1))
        scr = ctx.enter_context(tc.tile_pool(name="scr", bufs=2))
        small = ctx.enter_context(tc.tile_pool(name="small", bufs=1))

        # ---- inputs: X and its half-swapped copy (for the final residual) --
        X = big.tile([P, D], f32, name="X")
        nc.sync.dma_start(X[0:BC, :], xv_d[:])
        nc.scalar.dma_start(X[BC:P, :], xe_d[:])
        Xs = big.tile([P, D], f32, name="Xs")
        nc.sync.dma_start(Xs[BC:P, :], xv_d[:])
        nc.scalar.dma_start(Xs[0:BC, :], xe_d[:])
        coefs = small.tile([P, 2 * deg + 2], f32, name="coefs_t")
        nc.gpsimd.dma_start(coefs[:], coefs_d[:])

        # ---- dual power chains: P_k = X^k (for the output sum) and
        # Ps_k = Xs^k (for the swapped denominator + Y reductions). The
        # row-sums R_k of the X powers are exactly the coefficients the
        # SWAPPED denominator needs, so no cross-half moves are required. --
        R1t = small.tile([P, 1], f32, name="R1t")
        nc.vector.tensor_reduce(R1t[:], X[:], axis=mybir.AxisListType.X, op=ADD)
        R2t = small.tile([P, 1], f32, name="R2t")
        P2 = big.tile([P, D], f32, name="P2")
        nc.scalar.activation(P2[:], X[:], AF.Square, accum_out=R2t[:])
        P2s = big.tile([P, D], f32, name="P2s")
        nc.scalar.activation(P2s[:], Xs[:], AF.Square)
        R3t = small.tile([P, 1], f32, name="R3t")
        P3 = big.tile([P, D], f32, name="P3")
        nc.vector.scalar_tensor_tensor(
            out=P3[:], in0=P2[:], scalar=1.0, in1=X[:],
            op0=MULT, op1=MULT, accum_out=R3t[:],
        )
        R4t = small.tile([P, 1], f32, name="R4t")
        P4 = big.tile([P, D], f32, name="P4")
        nc.scalar.activation(P4[:], P2[:], AF.Square, accum_out=R4t[:])
        P3s = big.tile([P, D], f32, name="P3s")
        nc.vector.scalar_tensor_tensor(
            out=P3s[:], in0=P2s[:], scalar=1.0, in1=Xs[:], op0=MULT, op1=MULT,
        )
        P4s = big.tile([P, D], f32, name="P4s")
        nc.scalar.activation(P4s[:], P2s[:], AF.Square)

        # b_k = c_k * R_k (per-column, on GpSimd, unblocking as each R lands)
        Rts = {1: R1t, 2: R2t, 3: R3t, 4: R4t}
        Bts = {}
        for k in range(1, deg + 1):
            Bts[k] = small.tile([P, 1], f32, name=f"B{k}t")
            nc.gpsimd.tensor_tensor(
                out=Bts[k][:], in0=Rts[k][:], in1=coefs[:, k : k + 1], op=MULT,
            )

        # ---- den_s = swap(den) = cd0 + sum_k b_k Xs^k  (DVE chain) ----
        dB = scr.tile([P, D], f32, name="dB", tag="t")
        nc.vector.tensor_scalar(
            out=dB[:], in0=P2s[:], scalar1=Bts[2][:], scalar2=coefs[:, 0:1],
            op0=MULT, op1=ADD,
        )
        dA = scr.tile([P, D], f32, name="dA", tag="u")
        nc.vector.scalar_tensor_tensor(
            out=dA[:], in0=Xs[:], scalar=Bts[1][:], in1=dB[:],
            op0=MULT, op1=ADD,
        )
        dC = scr.tile([P, D], f32, name="dC", tag="t")
        nc.vector.scalar_tensor_tensor(
            out=dC[:], in0=P3s[:], scalar=Bts[3][:], in1=dA[:],
            op0=MULT, op1=ADD,
        )
        den = big.tile([P, D], f32, name="den")
        nc.vector.scalar_tensor_tensor(
            out=den[:], in0=P4s[:], scalar=Bts[4][:], in1=dC[:],
            op0=MULT, op1=ADD,
        )

        # ---- Y_ks = sum_j Xs^{k+1} / den_s  ( = swapped Y_k directly) ----
        rcp = big.tile([P, D], f32, name="rcp")
        nc.vector.reciprocal_approx_fast(out=rcp[:], in_=den[:])
        Pws = {1: Xs, 2: P2s, 3: P3s, 4: P4s}
        Gts = {}
        P5s = big.tile([P, D], f32, name="P5s")
        for k in range(0, deg + 1):
            if k == deg:
                # the highest swapped power, needed only by the last Y
                nc.vector.scalar_tensor_tensor(
                    out=P5s[:], in0=P4s[:], scalar=1.0, in1=Xs[:],
                    op0=MULT, op1=MULT,
                )
                Pws[deg + 1] = P5s
            q = scr.tile([P, D], f32, name=f"q{k}", tag="q")
            Yk = small.tile([P, 1], f32, name=f"Y{k}t")
            nc.vector.scalar_tensor_tensor(
                out=q[:], in0=Pws[k + 1][:], scalar=1.0, in1=rcp[:],
                op0=MULT, op1=MULT, accum_out=Yk[:],
            )
            # g_k = cout * c_k * Y_ks on GpSimd, fired per column
            Gts[k] = small.tile([P, 1], f32, name=f"G{k}t")
            nc.gpsimd.tensor_tensor(
                out=Gts[k][:], in0=Yk[:],
                in1=coefs[:, deg + 1 + k : deg + 2 + k], op=MULT,
            )

        # ---- OUT = swap(X) + g_0 + sum_k g_k X^k ----
        uA = scr.tile([P, D], f32, name="uA", tag="t")
        nc.scalar.activation(uA[:], X[:], AF.Identity,
                             bias=Gts[0][:], scale=Gts[1][:])
        uC = scr.tile([P, D], f32, name="uC", tag="u")
        nc.vector.scalar_tensor_tensor(
            out=uC[:], in0=P2[:], scalar=Gts[2][:], in1=uA[:],
            op0=MULT, op1=ADD,
        )
        uD = scr.tile([P, D], f32, name="uD", tag="t")
        nc.vector.scalar_tensor_tensor(
            out=uD[:], in0=P3[:], scalar=Gts[3][:], in1=Xs[:],
            op0=MULT, op1=ADD,
        )
        uE = scr.tile([P, D], f32, name="uE", tag="q")
        nc.vector.scalar_tensor_tensor(
            out=uE[:], in0=P4[:], scalar=Gts[4][:], in1=uD[:],
            op0=MULT, op1=ADD,
        )
        # split the final join so the first output DMAs fire earlier
        OUT = big.tile([P, D], f32, name="OUT")
        H = D // 2
        nc.vector.tensor_tensor(out=OUT[:, 0:H], in0=uC[:, 0:H],
                                in1=uE[:, 0:H], op=ADD)
        nc.sync.dma_start(ov_d[:, 0:H], OUT[BC:P, 0:H])
        nc.scalar.dma_start(oe_d[:, 0:H], OUT[0:BC, 0:H])
        nc.vector.tensor_tensor(out=OUT[:, H:D], in0=uC[:, H:D],
                                in1=uE[:, H:D], op=ADD)
        nc.sync.dma_start(ov_d[:, H:D], OUT[BC:P, H:D])
        nc.scalar.dma_start(oe_d[:, H:D], OUT[0:BC, H:D])

    nc.compile()
    return nc


_PROGRAMS: dict[int, object] = {}


def _get_program(deg: int):
    if deg not in _PROGRAMS:
        _PROGRAMS[deg] = _build_program(deg)
    return _PROGRAMS[deg]


def _host_constants(v, e, w_f, w_g, w_h, w_l, w_m, w_n, deg):
    alpha = float(np.dot(w_g.astype(np.float64), w_f.astype(np.float64)))
    beta = float(np.dot(w_h.astype(np.float64), w_m.astype(np.float64)))
    gamma = float(np.dot(w_l.astype(np.float64), w_n.astype(np.float64)))

    # per-batch bound on |s| = |alpha * e_i * v_j|
    m = abs(alpha) * float(
        (np.abs(e).max(axis=1) * np.abs(v).max(axis=1)).max()
    )
    m = max(m * 1.02, 1e-6)

    cheb = np.polynomial.chebyshev.Chebyshev.interpolate(np.exp, deg, domain=[-m, m])
    q = cheb.convert(kind=np.polynomial.polynomial.Polynomial).coef
    q = np.concatenate([q, np.zeros(deg + 1 - len(q))])
    c = np.array([q[k] * alpha**k for k in range(deg + 1)], dtype=np.float64)

    coefs = np.zeros((P, 2 * deg + 2), dtype=np.float32)
    coefs[:, 0] = c[0] * D
    coefs[:, 1 : deg + 1] = c[1:]
    # g-scale applies at the FINAL (already-swapped) position: the v-half
    # rows of OUT accumulate the e-side output (gamma), e-half beta.
    cout = np.where(np.arange(P) < BC, gamma, beta)
    for k in range(deg + 1):
        coefs[:, deg + 1 + k] = cout * c[k]
    return coefs


def _run(inputs: dict, trace: bool = False):
    v = np.ascontiguousarray(np.asarray(inputs["v_input"], dtype=np.float32))
    e = np.ascontiguousarray(np.asarray(inputs["e_input"], dtype=np.float32))
    assert v.shape == (B, D) and e.shape == (B, D), (v.shape, e.shape)
    ws = {k: np.asarray(inputs[k], dtype=np.float32)
          for k in ("w_f", "w_g", "w_h", "w_l", "w_m", "w_n")}

    coefs = _host_constants(
        v, e, ws["w_f"], ws["w_g"], ws["w_h"], ws["w_l"], ws["w_m"], ws["w_n"], DEG
    )

    nc = _get_program(DEG)
    in_maps = []
    for cidx in range(N_CORES):
        sl = slice(cidx * BC, (cidx + 1) * BC)
        in_maps.append(
            {
                "xv": np.ascontiguousarray(v[sl]),
                "xe": np.ascontiguousarray(e[sl]),
                "coefs": coefs,
            }
        )

    res = run_bass_kernel_spmd(nc, in_maps, list(range(N_CORES)), trace=trace)
    out_v = np.concatenate([res.results[c]["out_v"] for c in range(N_CORES)], axis=0)
    out_e = np.concatenate([res.results[c]["out_e"] for c in range(N_CORES)], axis=0)
    return (out_v, out_e), res


def kernel(**inputs):
    (out_v, out_e), _ = _run(inputs, trace=False)
    return out_v, out_e


# revision 16
# speedup vs baseline: 1.5104x; 1.1094x over previous
"""Trainium2 Bass kernel for the AttentionUnit GNN message-passing block.

Math
----
The nn.Module lifts scalars to `channel` dims with rank-1 weights, so the
whole block collapses to per-batch scalar attention:

    s[b,i,j] = alpha * e[b,i] * v[b,j],     alpha = w_g . w_f
    E = exp(s);  cs[j] = sum_i E[i,j];  rs[i] = sum_j E[i,j]
    out_v = v + beta  * E   @ (v / cs),     beta  = w_h . w_m
    out_e = e + gamma * E^T @ (e / rs),     gamma = w_l . w_n

Since |s| <= m ~ 0.3 (data-dependent, computed at runtime), exp(s) is
replaced by a degree-DEG Chebyshev-interpolated polynomial, which makes E a
rank-(DEG+1) matrix  E = sum_k c_k (e^k)(v^k)^T  that is never materialized:

    den = sum_k c_k A'_k X^k        (cs on the v-half, rs on the e-half)
    Y_k = sum_j X^{k+1} / den       (fused multiply-reduce against 1/den)
    out = swap(X) + sum_k g_k X^k   (g_k = swapped, scaled Y_k)

Layout: pure data parallel over 8 cores, 64 batch rows per core, stacked as
X = [v rows (partitions 0..63); e rows (64..127)] so every op handles both
sides at once. Cross-half swaps of [128,few] scalar blocks use two tiny
SBUF->SBUF DMAs; the final residual add uses a pre-swapped copy of the
input DMA'd at start, so no PE/transpose work is needed anywhere.

The polynomial coefficients depend on the input data, so they are passed as
small input tensors -> the compiled NEFF is input-independent and cached.
"""

import os
from contextlib import ExitStack

import numpy as np

import concourse.bass as bass
import concourse.tile as tile
from concourse import bacc, mybir
from concourse.bass_utils import run_bass_kernel_spmd

B = 512          # batch
D = 512          # dim
N_CORES = 8
BC = B // N_CORES  # 64 batch rows per core
P = 128            # partitions: [v (0..63); e (64..127)]
DEG = int(os.environ.get("ATTN_KERNEL_DEG", "4"))

f32 = mybir.dt.float32
MULT = mybir.AluOpType.mult
ADD = mybir.AluOpType.add
NCOL = 8  # padded column count for the R/Y scalar blocks
AF = mybir.ActivationFunctionType


def _build_program(deg: int):
    """Build + compile the single-core Tile program (same NEFF on all 8 cores)."""
    assert deg in (3, 4), "chains below are written for deg in {3, 4}"
    nc = bacc.Bacc(
        "TRN2",
        target_bir_lowering=False,
        debug=False,
        enable_asserts=False,
    )

    xv_d = nc.dram_tensor("xv", [BC, D], f32, kind="ExternalInput")
    xe_d = nc.dram_tensor("xe", [BC, D], f32, kind="ExternalInput")
    # coefs columns: [0] = c_0 * D
    #                [1 : deg+1]       = c_k (k=1..deg)            (den scale)
    #                [deg+1 : 2deg+2]  = swapped-cout * c_k (k=0..deg)
    coefs_d = nc.dram_tensor("coefs", [P, 2 * deg + 2], f32, kind="ExternalInput")
    ov_d = nc.dram_tensor("out_v", [BC, D], f32, kind="ExternalOutput")
    oe_d = nc.dram_tensor("out_e", [BC, D], f32, kind="ExternalOutput")

    with tile.TileContext(nc) as tc, ExitStack() as ctx:
        big = ctx.enter_context(tc.tile_pool(name="big", bufs=1))
        scr = ctx.enter_context(tc.tile_pool(name="scr", bufs=2))
        small = ctx.enter_context(tc.tile_pool(name="small", bufs=1))

        # ---- inputs: X and its half-swapped copy (for the final residual) --
        X = big.tile([P, D], f32, name="X")
        nc.sync.dma_start(X[0:BC, :], xv_d[:])
        nc.scalar.dma_start(X[BC:P, :], xe_d[:])
        Xs = big.tile([P, D], f32, name="Xs")
        nc.sync.dma_start(Xs[BC:P, :], xv_d[:])
        nc.scalar.dma_start(Xs[0:BC, :], xe_d[:])
        coefs = small.tile([P, 2 * deg + 2], f32, name="coefs_t")
        nc.gpsimd.dma_start(coefs[:], coefs_d[:])

        # ---- dual power chains: P_k = X^k (for the output sum) and
        # Ps_k = Xs^k (for the swapped denominator + Y reductions). The
        # row-sums R_k of the X powers are exactly the coefficients the
        # SWAPPED denominator needs, so no cross-half moves are required. --
        R1t = small.tile([P, 1], f32, name="R1t")
        nc.vector.tensor_reduce(R1t[:], X[:], axis=mybir.AxisListType.X, op=ADD)
        R2t = small.tile([P, 1], f32, name="R2t")
        P2 = big.tile([P, D], f32, name="P2")
        nc.scalar.activation(P2[:], X[:], AF.Square, accum_out=R2t[:])
        P2s = big.tile([P, D], f32, name="P2s")
        nc.scalar.activation(P2s[:], Xs[:], AF.Square)
        R3t = small.tile([P, 1], f32, name="R3t")
        P3 = big.tile([P, D], f32, name="P3")
        nc.vector.scalar_tensor_tensor(
            out=P3[:], in0=P2[:], scalar=1.0, in1=X[:],
            op0=MULT, op1=MULT, accum_out=R3t[:],
        )
        Rts = {1: R1t, 2: R2t, 3: R3t}
        Pw = {1: X, 2: P2, 3: P3}
        if deg >= 4:
            R4t = small.tile([P, 1], f32, name="R4t")
            P4 = big.tile([P, D], f32, name="P4")
            nc.scalar.activation(P4[:], P2[:], AF.Square, accum_out=R4t[:])
            Rts[4] = R4t
            Pw[4] = P4
        P3s = big.tile([P, D], f32, name="P3s")
        nc.vector.scalar_tensor_tensor(
            out=P3s[:], in0=P2s[:], scalar=1.0, in1=Xs[:], op0=MULT, op1=MULT,
        )
        P4s = big.tile([P, D], f32, name="P4s")
        nc.scalar.activation(P4s[:], P2s[:], AF.Square)
        Pws = {1: Xs, 2: P2s, 3: P3s, 4: P4s}

        # b_k = c_k * R_k (per-column, on GpSimd, unblocking as each R lands)
        Bts = {}
        for k in range(1, deg + 1):
            Bts[k] = small.tile([P, 1], f32, name=f"B{k}t")
            nc.gpsimd.tensor_tensor(
                out=Bts[k][:], in0=Rts[k][:], in1=coefs[:, k : k + 1], op=MULT,
            )

        # ---- den_s = swap(den) = cd0 + sum_k b_k Xs^k  (DVE chain) ----
        dB = scr.tile([P, D], f32, name="dB", tag="t")
        nc.vector.tensor_scalar(
            out=dB[:], in0=P2s[:], scalar1=Bts[2][:], scalar2=coefs[:, 0:1],
            op0=MULT, op1=ADD,
        )
        dA = scr.tile([P, D], f32, name="dA", tag="u")
        nc.vector.scalar_tensor_tensor(
            out=dA[:], in0=Xs[:], scalar=Bts[1][:], in1=dB[:],
            op0=MULT, op1=ADD,
        )
        dprev = dA
        for k in range(3, deg + 1):
            dnx = scr.tile([P, D], f32, name=f"d{k}", tag="t" if k % 2 else "u")
            nc.vector.scalar_tensor_tensor(
                out=dnx[:], in0=Pws[k][:], scalar=Bts[k][:], in1=dprev[:],
                op0=MULT, op1=ADD,
            )
            dprev = dnx
        den = dprev

        # ---- Y_ks = sum_j Xs^{k+1} / den_s  ( = swapped Y_k directly) ----
        rcp = big.tile([P, D], f32, name="rcp")
        nc.vector.reciprocal_approx_fast(out=rcp[:], in_=den[:])
        Gts = {}
        for k in range(0, deg + 1):
            if k + 1 not in Pws:
                # the highest swapped power, needed only by the last Y
                Ptop = big.tile([P, D], f32, name=f"P{k + 1}s")
                nc.vector.scalar_tensor_tensor(
                    out=Ptop[:], in0=Pws[k][:], scalar=1.0, in1=Xs[:],
                    op0=MULT, op1=MULT,
                )
                Pws[k + 1] = Ptop
            q = scr.tile([P, D], f32, name=f"q{k}", tag="q")
            Yk = small.tile([P, 1], f32, name=f"Y{k}t")
            nc.vector.scalar_tensor_tensor(
                out=q[:], in0=Pws[k + 1][:], scalar=1.0, in1=rcp[:],
                op0=MULT, op1=MULT, accum_out=Yk[:],
            )
            # g_k = cout * c_k * Y_ks on GpSimd, fired per column
            Gts[k] = small.tile([P, 1], f32, name=f"G{k}t")
            nc.gpsimd.tensor_tensor(
                out=Gts[k][:], in0=Yk[:],
                in1=coefs[:, deg + 1 + k : deg + 2 + k], op=MULT,
            )

        # ---- OUT = swap(X) + g_0 + sum_k g_k X^k ----
        uA = scr.tile([P, D], f32, name="uA", tag="t")
        nc.scalar.activation(uA[:], X[:], AF.Identity,
                             bias=Gts[0][:], scale=Gts[1][:])
        uC = scr.tile([P, D], f32, name="uC", tag="u")
        nc.vector.scalar_tensor_tensor(
            out=uC[:], in0=P2[:], scalar=Gts[2][:], in1=uA[:],
            op0=MULT, op1=ADD,
        )
        # second branch (from Xs) and the final join+DMA, split by free-dim
        # halves so the first output DMAs fire while the second half computes
        OUT = big.tile([P, D], f32, name="OUT")
        H = D // 2
        dma_eng = [(nc.sync, nc.scalar), (nc.gpsimd, nc.sync)]
        for h, (engA, engB) in enumerate(dma_eng):
            sl = slice(h * H, (h + 1) * H)
            zprev = None
            for k in range(3, deg + 1):
                znx = scr.tile([P, H], f32, name=f"z{k}h{h}", tag="zh")
                nc.vector.scalar_tensor_tensor(
                    out=znx[:], in0=Pw[k][:, sl], scalar=Gts[k][:],
                    in1=(Xs[:, sl] if zprev is None else zprev[:]),
                    op0=MULT, op1=ADD,
                )
                zprev = znx
            zsl = Xs[:, sl] if zprev is None else zprev[:]
            nc.vector.tensor_tensor(out=OUT[:, sl], in0=uC[:, sl],
                                    in1=zsl, op=ADD)
            engA.dma_start(ov_d[:, sl], OUT[BC:P, sl])
            engB.dma_start(oe_d[:, sl], OUT[0:BC, sl])

    nc.compile()
    return nc


_PROGRAMS: dict[int, object] = {}


def _get_program(deg: int):
    if deg not in _PROGRAMS:
        _PROGRAMS[deg] = _build_program(deg)
    return _PROGRAMS[deg]


def _host_constants(v, e, w_f, w_g, w_h, w_l, w_m, w_n, deg):
    alpha = float(np.dot(w_g.astype(np.float64), w_f.astype(np.float64)))
    beta = float(np.dot(w_h.astype(np.float64), w_m.astype(np.float64)))
    gamma = float(np.dot(w_l.astype(np.float64), w_n.astype(np.float64)))

    # per-batch bound on |s| = |alpha * e_i * v_j|
    m = abs(alpha) * float(
        (np.abs(e).max(axis=1) * np.abs(v).max(axis=1)).max()
    )
    m = max(m * 1.02, 1e-6)

    cheb = np.polynomial.chebyshev.Chebyshev.interpolate(np.exp, deg, domain=[-m, m])
    q = cheb.convert(kind=np.polynomial.polynomial.Polynomial).coef
    q = np.concatenate([q, np.zeros(deg + 1 - len(q))])
    c = np.array([q[k] * alpha**k for k in range(deg + 1)], dtype=np.float64)

    coefs = np.zeros((P, 2 * deg + 2), dtype=np.float32)
    coefs[:, 0] = c[0] * D
    coefs[:, 1 : deg + 1] = c[1:]
    # g-scale applies at the FINAL (already-swapped) position: the v-half
    # rows of OUT accumulate the e-side output (gamma), e-half beta.
    cout = np.where(np.arange(P) < BC, gamma, beta)
    for k in range(deg + 1):
        coefs[:, deg + 1 + k] = cout * c[k]
    return coefs


def _run(inputs: dict, trace: bool = False):
    v = np.ascontiguousarray(np.asarray(inputs["v_input"], dtype=np.float32))
    e = np.ascontiguousarray(np.asarray(inputs["e_input"], dtype=np.float32))
    assert v.shape == (B, D) and e.shape == (B, D), (v.shape, e.shape)
    ws = {k: np.asarray(inputs[k], dtype=np.float32)
          for k in ("w_f", "w_g", "w_h", "w_l", "w_m", "w_n")}

    coefs = _host_constants(
        v, e, ws["w_f"], ws["w_g"], ws["w_h"], ws["w_l"], ws["w_m"], ws["w_n"], DEG
    )

    nc = _get_program(DEG)
    in_maps = []
    for cidx in range(N_CORES):
        sl = slice(cidx * BC, (cidx + 1) * BC)
        in_maps.append(
            {
                "xv": np.ascontiguousarray(v[sl]),
                "xe": np.ascontiguousarray(e[sl]),
                "coefs": coefs,
            }
        )

    res = run_bass_kernel_spmd(nc, in_maps, list(range(N_CORES)), trace=trace)
    out_v = np.concatenate([res.results[c]["out_v"] for c in range(N_CORES)], axis=0)
    out_e = np.concatenate([res.results[c]["out_e"] for c in range(N_CORES)], axis=0)
    return (out_v, out_e), res


def kernel(**inputs):
    (out_v, out_e), _ = _run(inputs, trace=False)
    return out_v, out_e
